# revision 22
# baseline (speedup 1.0000x reference)
"""Trainium2 Bass kernel for nn_CrossAttentionWithBias.

Multi-head cross attention with additive bias + key padding mask:
    q = query @ Wq + bq; k = key @ Wk + bk; v = value @ Wv + bv   (per-head split)
    logits = q k^T * hd^-0.5 + attn_bias;  masked keys -> -inf
    out = softmax(logits) @ v, heads merged, @ Wo + bo

Sharding: 8 NeuronCores; core c handles batch b = c//4 and the 4 heads
[4*(c%4), 4*(c%4)+4). Each core computes a partial of the output projection
(its 256 hd-columns' contribution); the host sums the 4 partials per batch
and adds bo.

On-core layout (all "transposed"): activations are stored d-major so the
contraction dim of every matmul is the partition dim:
    qT/kT/vT  [D, n]        (host-transposed, bf16)
    Q^T,K^T   [hd=256, n]   = W.T @ actT     (PE)
    V         [nk, hd]      (natural, PE)    + an all-ones column per head
    S^T       [nk, nq] = K^T.T @ Q^T per head; bias^T folded via exp-decompose
    AV^T      [65, nq] accumulated over 16 k-tiles; row 64 = softmax denom
              (from the ones column) -> reciprocal -> broadcast -> scale
    partial   = AV^T.T @ Wo[hd-rows]         (PE), bf16 out
Softmax skips the max-subtraction: logits here are O(+-10) and masked
entries are -1e30 (exp -> 0 exactly), so exp cannot overflow in f32.

Schedule (vs the 194us baseline): input DMAs are split per d-tile and
issued weights-first on the scalar HWDGE queue so the projections (kt-outer)
trail the DMA stream instead of waiting for it; the attention exp stream
(the ACT-engine wall, ~73us) starts as soon as Q^T/K^T exist; V-projection
passes and AV matmuls of earlier heads are emission-interleaved with S
matmuls of later heads so the in-order PE queue never starves ACT.
"""

import numpy as np

B, NQ, NK, D, H = 2, 1024, 2048, 1024, 16
HD = D // H          # 64
SCALE = HD ** -0.5
N_CORES = 8
CORES_PER_B = N_CORES // B   # 4
HPC = H // CORES_PER_B       # heads per core = 4
HDC = HPC * HD               # hd-columns per core = 256
P = 128
KT_N = NK // P               # 16 k-tiles
DT = D // P                  # 8 d-tiles
QT_N = NQ // P               # 8 q-tiles

BF16 = np.float16
NEG = np.float32(-1e30)
TIMER_NOP = 600          # gpsimd timed-nop cycles per timer tick (1.2 GHz)


# ----------------------------------------------------------------- device ---

def _build_nc(reps: int = 1, timing: bool = False, timer_nop: int = TIMER_NOP):
    # timing=True: big tensors become Internal DRAM (garbage contents; engine
    # timing is data-independent) so per-call host<->device transfer is tiny
    # and wall-clock deltas measure device execution.
    import concourse.bacc as bacc
    import concourse.mybir as mybir
    import concourse.tile as tile

    f32 = mybir.dt.float32
    bf16 = mybir.dt.float16  # "bf16" name kept; fp16 has 11-bit mantissa at same byte width
    Exp = mybir.ActivationFunctionType.Exp
    Ident = mybir.ActivationFunctionType.Identity
    MULT = mybir.AluOpType.mult

    nc = bacc.Bacc("TRN2", target_bir_lowering=False, debug=False,
                   num_devices=N_CORES)

    ikind = "Internal" if timing else "ExternalInput"
    okind = "Internal" if timing else "ExternalOutput"
    qT = nc.dram_tensor("qT", [D, NQ], bf16, kind=ikind).ap()
    kT = nc.dram_tensor("kT", [D, NK], bf16, kind=ikind).ap()
    vT = nc.dram_tensor("vT", [D, NK], bf16, kind=ikind).ap()
    wq = nc.dram_tensor("wq", [D, HDC], bf16, kind=ikind).ap()
    wk = nc.dram_tensor("wk", [D, HDC], bf16, kind=ikind).ap()
    wv = nc.dram_tensor("wv", [D, HDC], bf16, kind=ikind).ap()
    wo = nc.dram_tensor("wo", [HDC, D], bf16, kind=ikind).ap()
    bq = nc.dram_tensor("bq", [HDC, 1], f32, kind=ikind).ap()
    bk = nc.dram_tensor("bk", [HDC, 1], f32, kind=ikind).ap()
    bv = nc.dram_tensor("bv", [1, HDC], f32, kind=ikind).ap()
    biasT = nc.dram_tensor("biasT", [HPC, NK, NQ], bf16, kind=ikind).ap()
    part = nc.dram_tensor("part", [NQ, D], bf16, kind=okind).ap()
    if timing:
        tin = nc.dram_tensor("tin", [1, P], f32, kind="ExternalInput").ap()
        tout = nc.dram_tensor("tout", [1, P], f32, kind="ExternalOutput").ap()

    qTr = qT.rearrange("(t p) n -> p t n", p=P)
    kTr = kT.rearrange("(t p) n -> p t n", p=P)
    vTr = vT.rearrange("(t p) n -> p t n", p=P)

    with tile.TileContext(nc) as tc:
        from contextlib import ExitStack
        with ExitStack() as ctx:
            wpool = ctx.enter_context(tc.tile_pool(name="w", bufs=1))
            # PSUM partition (8 banks total):
            #   sp  2 slots x 2 banks: Q/K projection psums, attention s
            #       tiles, O-proj psums (rotation doubles as s double-buffer)
            #   avp 1 slot  x 2 banks: av accumulators (heads sequential)
            #   vp  2 slots x 1 bank : V-projection, one nk-tile per slot
            sp = ctx.enter_context(tc.tile_pool(name="sp", bufs=2, space="PSUM"))
            avp = ctx.enter_context(tc.tile_pool(name="avp", bufs=1, space="PSUM"))
            vp = ctx.enter_context(tc.tile_pool(name="vp", bufs=2, space="PSUM"))
            apool = ctx.enter_context(tc.tile_pool(name="a", bufs=1))
            dpool = ctx.enter_context(tc.tile_pool(name="d", bufs=2))
            bias_pool = ctx.enter_context(tc.tile_pool(name="bias", bufs=4))
            e_pool = ctx.enter_context(tc.tile_pool(name="e", bufs=3))
            et_pool = ctx.enter_context(tc.tile_pool(name="et", bufs=4))
            small = ctx.enter_context(tc.tile_pool(name="small", bufs=2))
            ostage = ctx.enter_context(tc.tile_pool(name="ostage", bufs=4))

            if timing:
                # On-device timer: the sync sequencer spins on a timed-nop
                # loop, polling an SBUF flag via an aliased handle (invisible
                # to the dep tracker, so the spin doesn't deadlock against
                # the final flag write). count * period = kernel span.
                tmr_pool = ctx.enter_context(tc.tile_pool(name="tmr", bufs=1))
                cnt_sb = tmr_pool.tile([1, 2], mybir.dt.int32, tag="cnt")
                wflag = nc.alloc_sbuf_tensor("wflag", [1, 1], f32)
                waddr = nc.lookup_mloc(wflag).addr
                rflag = nc.alloc_sbuf_tensor_at(
                    "rflag", [1, 1], mybir.dt.int32, offset=waddr)
                with tc.tile_critical():
                    g = nc.sync
                    g.store(rflag.ap()[0:1, 0:1], 0)
                    with g.register("tmr_cnt") as cnt, \
                         g.register("tmr_f") as fr, \
                         g.register("tmr_c") as cr:
                        g.reg_mov(cnt, 0)
                        g.reg_mov(cr, 1)
                        with g.While(lambda: cr):
                            g.nop(cycle_cnt=timer_nop)
                            g.reg_alu(cnt, cnt, 1, mybir.AluOpType.add)
                            g.reg_load(fr, rflag.ap()[0:1, 0:1])
                            g.reg_alu(cr, fr, 0, mybir.AluOpType.is_equal)
                        g.store(cnt_sb[0:1, 0:1], cnt)

            # V (+ 64 ones cols -> replicated softmax denom): [P, kt, h, 128]
            # allocated once; ones written once, V cols rewritten per rep
            V_sb = apool.tile([P, KT_N, HPC, 2 * HD], bf16, tag="V")
            nc.vector.memset(V_sb[:, :, :, HD:], 1.0)

            if timing:
                # Internal DRAM holds garbage; fill with benign values so the
                # compute stays finite (exp(0)=1, biasT=1 -> denom=NK).
                zfill = apool.tile([P, 2048], bf16, tag="zfill")
                nc.vector.memset(zfill[:], 0.0)
                ofill = apool.tile([P, 2048], bf16, tag="ofill")
                nc.vector.memset(ofill[:], 1.0)
                zf32 = apool.tile([P, 2 * HDC], f32, tag="zf32")
                nc.vector.memset(zf32[:], 0.0)
                for t, width in ((qT, NQ), (kT, NK), (vT, NK)):
                    r = t.rearrange("(t p) n -> p t n", p=P)
                    for i in range(DT):
                        for j in range(width // 2048):
                            nc.scalar.dma_start(
                                r[:, i, j * 2048:(j + 1) * 2048],
                                zfill[:, 0:2048])
                        if width < 2048:
                            nc.scalar.dma_start(r[:, i, :], zfill[:, 0:width])
                for t in (wq, wk, wv):
                    r = t.rearrange("(t p) m -> p t m", p=P)
                    for i in range(DT):
                        nc.scalar.dma_start(r[:, i, :], zfill[:, 0:HDC])
                r = wo.rearrange("(t p) n -> p t n", p=P)
                for i in range(HDC // P):
                    nc.scalar.dma_start(r[:, i, :], zfill[:, 0:D])
                nc.scalar.dma_start(bq.rearrange("(t p) o -> p t o", p=P),
                                  zf32[:, 0:2].bitcast(f32)[:, 0:2, None])
                nc.scalar.dma_start(bk.rearrange("(t p) o -> p t o", p=P),
                                  zf32[:, 0:2].bitcast(f32)[:, 0:2, None])
                nc.scalar.dma_start(bv[:], zf32[0:1, 0:HDC])
                for h in range(HPC):
                    r = biasT[h].rearrange("(t p) q -> p t q", p=P)
                    for i in range(KT_N):
                        nc.scalar.dma_start(r[:, i, :], ofill[:, 0:NQ])

            for _ in range(reps):
                # ---- input DMAs on the scalar HWDGE queue (FIFO per
                # engine) so arrival order == consumption order.  Stream:
                # wv,bv, vT nk-half0 | wq,bq,qT | wk,bk,kT | vT half1 | wo.
                # vT half0 rides first so V nk-tiles 0-7 exist before the
                # attention consumes them; half1 lands while exp runs. ----
                wv_sb = wpool.tile([P, DT, HDC], bf16, tag="wv")
                nc.scalar.dma_start(wv_sb[:], wv.rearrange("(t p) m -> p t m", p=P))
                bv_sb = wpool.tile([P, HDC], f32, tag="bv")
                nc.scalar.dma_start(bv_sb[:], bv[:].to_broadcast((P, HDC)))
                bv_bc = bv_sb[:].rearrange("p (h e) -> p h e", h=HPC)
                vT_sb = apool.tile([P, DT, NK], bf16, tag="vT")
                for t in range(DT):
                    nc.scalar.dma_start(vT_sb[:, t, :], vTr[:, t, :])
                wq_sb = wpool.tile([P, DT, HDC], bf16, tag="wq")
                nc.scalar.dma_start(wq_sb[:], wq.rearrange("(t p) m -> p t m", p=P))
                bq_sb = wpool.tile([P, HDC // P, 1], f32, tag="bq")
                nc.scalar.dma_start(bq_sb[:], bq.rearrange("(t p) o -> p t o", p=P))
                qT_sb = apool.tile([P, DT, NQ], bf16, tag="qT")
                for t in range(DT):
                    nc.scalar.dma_start(qT_sb[:, t, :], qTr[:, t, :])
                wk_sb = wpool.tile([P, DT, HDC], bf16, tag="wk")
                nc.scalar.dma_start(wk_sb[:], wk.rearrange("(t p) m -> p t m", p=P))
                bk_sb = wpool.tile([P, HDC // P, 1], f32, tag="bk")
                nc.scalar.dma_start(bk_sb[:], bk.rearrange("(t p) o -> p t o", p=P))
                kT_sb = apool.tile([P, DT, NK], bf16, tag="kT")
                for t in range(DT):
                    nc.scalar.dma_start(kT_sb[:, t, :], kTr[:, t, :])
                wo_sb = wpool.tile([P, HDC // P, D], bf16, tag="wo")

                # QT/KT: [hd, n] as [P, ht, n];  AVT result shares layout
                QT_sb = dpool.tile([P, 2, NQ], bf16, tag="QT")
                KT_sb = dpool.tile([P, 2, NK], bf16, tag="KT")
                AVT_sb = dpool.tile([P, 2, NQ], bf16, tag="AVT")

                # V-projection of one nk-tile mt into a 1-bank vp slot;
                # d_tiles=None -> full d-inner contraction in one call.
                psV = [None] * KT_N

                def v_mm(mt, d_tiles=None, evac=False):
                    if psV[mt] is None:
                        psV[mt] = vp.tile([P, 512], f32, tag="psV",
                                          name="psV")
                    for kt in (range(DT) if d_tiles is None else d_tiles):
                        nc.tensor.matmul(
                            psV[mt][:, 0:HDC],
                            lhsT=vT_sb[:, kt, mt * P:(mt + 1) * P],
                            rhs=wv_sb[:, kt, :],
                            start=(kt == 0), stop=(kt == DT - 1))
                    if evac or d_tiles is None:
                        nc.vector.tensor_add(
                            V_sb[:, mt, :, 0:64],
                            psV[mt][:, 0:HDC].rearrange("p (h e) -> p h e",
                                                        h=HPC),
                            bv_bc)

                # ---- V nk-tiles 0,1 d-outer (trail the vT stream),
                # then 2..15 d-inner (vT resident); all pre-attention ----
                for kt in range(DT):
                    v_mm(0, d_tiles=[kt], evac=(kt == DT - 1))
                    v_mm(1, d_tiles=[kt], evac=(kt == DT - 1))
                for mt in range(2, KT_N):
                    v_mm(mt)

                # ---- Q projection, kt-outer (trails the qT DMA stream) ----
                psQ = [sp.tile([P, NQ], f32, tag="ps", name="psQ") for _ in range(2)]
                for kt in range(DT):
                    for ht in range(2):
                        for ch in range(2):
                            csl = slice(ch * 512, (ch + 1) * 512)
                            nc.tensor.matmul(
                                psQ[ht][:, csl],
                                lhsT=wq_sb[:, kt, ht * P:(ht + 1) * P],
                                rhs=qT_sb[:, kt, csl],
                                start=(kt == 0), stop=(kt == DT - 1))
                for ht in range(2):
                    nc.vector.tensor_scalar(
                        QT_sb[:, ht, :], psQ[ht][:], bq_sb[:, ht, :], None,
                        mybir.AluOpType.add)

                # ---- K projection, nk-quarter-major in 1-bank vp slots.
                # Needs the full kT stream (d-contraction), so quarter 0 of
                # ht0 gates the first S; the other quarters are injected
                # into attention loops ahead of their first use. ----
                psKq_live = {}

                def k_quarter(ht, q, part_i=None):
                    # part_i 0/1 splits the 8-MM contraction into two bursts
                    # (injected at adjacent attention steps to stay inside
                    # the per-step PE slack); None = both at once.
                    qsl = slice(q * 512, (q + 1) * 512)
                    if part_i in (0, None):
                        psKq_live[(ht, q)] = vp.tile([P, 512], f32,
                                                     tag="psV", name="psK")
                    psKq = psKq_live[(ht, q)]
                    tiles = (range(DT) if part_i is None else
                             range(part_i * DT // 2, (part_i + 1) * DT // 2))
                    for kt in tiles:
                        nc.tensor.matmul(
                            psKq[:], lhsT=wk_sb[:, kt, ht * P:(ht + 1) * P],
                            rhs=kT_sb[:, kt, qsl],
                            start=(kt == 0), stop=(kt == DT - 1))
                    if part_i in (1, None):
                        nc.vector.tensor_scalar(
                            KT_sb[:, ht, qsl], psKq[:], bk_sb[:, ht, :],
                            None, mybir.AluOpType.add)

                k_quarter(0, 0)

                # Gate the gpsimd-side DMAs (eb stream, wo) behind the kT
                # DMAs so their prefetch doesn't steal HBM bandwidth from
                # the input stream that gates the first exp.  The Tile
                # scheduler runs any *ready* instruction past a blocked one,
                # so ordering alone is not enough: each gated DMA's buffer
                # first gets a corner written from gate_sb (which depends on
                # kT), making the DMA itself transitively depend on kT.
                gate_sb = small.tile([1, 8], bf16, tag="gate")
                nc.gpsimd.tensor_copy(gate_sb[:], kT_sb[0:1, DT - 1, 0:8])

                def gated_pool_dma(dst_tile, corner, src_ap):
                    nc.gpsimd.tensor_copy(corner, gate_sb[0:1, 0:1])
                    nc.gpsimd.dma_start(dst_tile, src_ap)

                gated_pool_dma(wo_sb[:], wo_sb[0:1, 0, 0:1],
                               wo.rearrange("(t p) n -> p t n", p=P))

                # ---- attention ----
                # biasT input holds exp(bias^T + mask) ("EB"); logits
                # exp-decompose: exp(S+B) = exp(S) * EB, so the bias lands
                # as a 2-byte DVE multiply (2x mode) off the PSUM path.
                # Per head, per k-tile: S (PE) -> exp (ACT) -> *EB (DVE) ->
                # AV accumulate (PE).  s tiles rotate through sp's 2 slots,
                # so PE runs one k-tile ahead of the exp; the ACT exp stream
                # (the wall) never waits.  V nk-tiles 8-15 (vT half1, which
                # lands mid-attention) are injected into h0's loop at spots
                # where the PE would idle anyway, ahead of their AV use.
                eb_tiles = {}

                def attn_head(h, inject):
                    ht, hp = h // 2, (h % 2) * HD
                    av = avp.tile([P, NQ], f32, tag="av")
                    for kt in range(KT_N):
                        if kt % 2 == 0:
                            eb = bias_pool.tile([P, 2, NQ], bf16, tag="bias")
                            gated_pool_dma(
                                eb[:], eb[0:1, 0, 0:1],
                                biasT[h].rearrange("(t p) q -> p t q",
                                                   p=P)[:, kt:kt + 2, :])
                            eb_tiles[h] = eb
                        s = sp.tile([P, NQ], f32, tag="ps", name="s")
                        ksl = slice(kt * P, (kt + 1) * P)
                        for ch in range(2):
                            csl = slice(ch * 512, (ch + 1) * 512)
                            nc.tensor.matmul(
                                s[:, csl],
                                lhsT=KT_sb[hp:hp + HD, ht, ksl],
                                rhs=QT_sb[hp:hp + HD, ht, csl],
                                start=True, stop=True)
                        e0 = e_pool.tile([P, NQ], bf16, tag="e")
                        nc.scalar.activation(e0[:], s[:], Exp)
                        et = et_pool.tile([P, NQ], bf16, tag="et")
                        nc.vector.tensor_tensor(
                            et[:], e0[:], eb_tiles[h][:, kt % 2, :], op=MULT)
                        for ch in range(2):
                            csl = slice(ch * 512, (ch + 1) * 512)
                            nc.tensor.matmul(
                                av[:, csl],
                                lhsT=V_sb[:, kt, h, :],
                                rhs=et[:, csl],
                                start=(kt == 0), stop=(kt == KT_N - 1))
                        for fn in inject.get(kt, ()):
                            fn()
                    rec_bc = small.tile([HD, NQ], f32, tag="rec_bc")
                    nc.vector.reciprocal(rec_bc[:], av[HD:2 * HD, :])
                    nc.vector.tensor_tensor(
                        AVT_sb[hp:hp + HD, ht, :], av[0:HD, :],
                        rec_bc[:], op=MULT)

                # Injected K-quarter work, placed ahead of its first
                # consumer: ht0 quarters 1-3 before S_h0 k-tiles 4/8/12;
                # ht1 quarters before exp_h2 k-tiles 0/4/8/12.
                attn_head(0, {
                    0: (lambda: k_quarter(0, 1, 0),),
                    1: (lambda: k_quarter(0, 1, 1),),
                    2: (lambda: k_quarter(0, 2, 0),),
                    3: (lambda: k_quarter(0, 2, 1),),
                    4: (lambda: k_quarter(0, 3, 0),),
                    5: (lambda: k_quarter(0, 3, 1),),
                })
                attn_head(1, {
                    0: (lambda: k_quarter(1, 0, 0),),
                    1: (lambda: k_quarter(1, 0, 1),),
                    4: (lambda: k_quarter(1, 1, 0),),
                    5: (lambda: k_quarter(1, 1, 1),),
                })
                attn_head(2, {
                    1: (lambda: k_quarter(1, 2, 0),),
                    2: (lambda: k_quarter(1, 2, 1),),
                    5: (lambda: k_quarter(1, 3, 0),),
                    6: (lambda: k_quarter(1, 3, 1),),
                })
                attn_head(3, {})

                # ---- output projection (partial), bf16 out ----
                last_ot = None
                for mt in range(QT_N):
                    ps = sp.tile([P, D], f32, tag="ps", name="ops")
                    for ch in range(D // 512):
                        csl = slice(ch * 512, (ch + 1) * 512)
                        for kt in range(HDC // P):
                            nc.tensor.matmul(
                                ps[:, csl],
                                lhsT=AVT_sb[:, kt, mt * P:(mt + 1) * P],
                                rhs=wo_sb[:, kt, csl],
                                start=(kt == 0), stop=(kt == HDC // P - 1))
                    ot = ostage.tile([P, D], bf16, tag="ot")
                    nc.vector.tensor_copy(ot[:, 0:512], ps[:, 0:512])
                    nc.scalar.copy(ot[:, 512:D], ps[:, 512:D])
                    nc.gpsimd.dma_start(
                        part[mt * P:(mt + 1) * P, :], ot[:])
                    last_ot = ot
            if timing:
                # tiny externally-visible dependency so the compute isn't dead
                tsb = small.tile([1, P], f32, tag="tsb")
                nc.scalar.dma_start(tsb[:], tin[:])
                nc.vector.tensor_add(tsb[:], tsb[:], last_ot[0:1, 0:P])
                # stop the timer: write nonzero to the flag (depends on tsb)
                nc.vector.tensor_scalar(
                    wflag.ap()[0:1, 0:1], tsb[0:1, 0:1], 0.0, 1.0,
                    mybir.AluOpType.mult, mybir.AluOpType.add)
                nc.scalar.dma_start(tout[:], tsb[:])
                nc.scalar.dma_start(tout[0:1, 0:2].bitcast(mybir.dt.int32),
                                    cnt_sb[0:1, 0:2])

    nc.compile()
    return nc


# ----------------------------------------------------------------- runner ---

def _make_runner(nc, n_cores):
    import jax
    from jax.sharding import Mesh, PartitionSpec
    from jax.experimental.shard_map import shard_map
    import concourse.mybir as mybir
    from concourse import bass2jax
    from concourse.bass2jax import _bass_exec_p, install_neuronx_cc_hook

    install_neuronx_cc_hook()
    partition_name = nc.partition_id_tensor.name if nc.partition_id_tensor else None
    dbg_name = nc.dbg_addr.name if nc.dbg_addr is not None else None
    in_names, out_names, out_avals, zero_outs = [], [], [], []
    for alloc in nc.m.functions[0].allocations:
        if not isinstance(alloc, mybir.MemoryLocationSet):
            continue
        name = alloc.memorylocations[0].name
        if alloc.kind == "ExternalInput":
            if name not in (partition_name, dbg_name):
                in_names.append(name)
        elif alloc.kind == "ExternalOutput":
            out_names.append(name)
            shape = tuple(alloc.tensor_shape)
            dtype = mybir.dt.np(alloc.dtype)
            out_avals.append(jax.core.ShapedArray(shape, dtype))
            zero_outs.append(np.zeros(shape, dtype))
    n_params = len(in_names)
    all_in_names = list(in_names) + list(out_names)
    if dbg_name is not None:
        all_in_names.append(dbg_name)
    if partition_name is not None:
        all_in_names.append(partition_name)

    def _body(*args):
        operands = list(args)
        if dbg_name is not None:
            operands.append(np.zeros((1, 2), np.uint32))
        if partition_name is not None:
            operands.append(bass2jax.partition_id_tensor())
        outs = _bass_exec_p.bind(
            *operands,
            out_avals=tuple(out_avals),
            in_names=tuple(all_in_names),
            out_names=tuple(out_names),
            lowering_input_output_aliases=(),
            sim_require_finite=True,
            sim_require_nnan=True,
            nc=nc,
        )
        return tuple(outs)

    devices = jax.devices()[:n_cores]
    mesh = Mesh(np.asarray(devices), ("core",))
    in_specs = (PartitionSpec("core"),) * (n_params + len(out_names))
    out_specs = (PartitionSpec("core"),) * len(out_names)
    sharded = jax.jit(
        shard_map(_body, mesh=mesh, in_specs=in_specs, out_specs=out_specs,
                  check_rep=False),
        keep_unused=True,
    )

    def run(in_maps):
        concat_in = [
            np.concatenate([np.asarray(in_maps[c][n]) for c in range(n_cores)],
                           axis=0)
            for n in in_names
        ]
        concat_zeros = [
            np.zeros((n_cores * z.shape[0], *z.shape[1:]), z.dtype)
            for z in zero_outs
        ]
        out = sharded(*concat_in, *concat_zeros)
        return [
            {name: np.asarray(out[i]).reshape(n_cores, *out_avals[i].shape)[c]
             for i, name in enumerate(out_names)}
            for c in range(n_cores)
        ]

    run.sharded = sharded
    run.in_names = in_names
    run.out_names = out_names
    run.out_avals = out_avals
    run.zero_outs = zero_outs
    return run


_CACHE = {}


def get_runner(reps: int = 1, timing: bool = False, timer_nop: int = TIMER_NOP):
    key = (reps, timing, timer_nop)
    if key not in _CACHE:
        nc = _build_nc(reps, timing=timing, timer_nop=timer_nop)
        _CACHE[key] = _make_runner(nc, N_CORES)
    return _CACHE[key]


# ------------------------------------------------------------------- host ---

def make_in_maps(query, key, value, attn_bias, key_padding_mask,
                 Wq, bq, Wk, bk, Wv, bv, Wo, bo):
    query = np.asarray(query, np.float32)
    key = np.asarray(key, np.float32)
    value = np.asarray(value, np.float32)
    attn_bias = np.asarray(attn_bias, np.float32)
    mask = np.asarray(key_padding_mask, bool)
    Wq = np.asarray(Wq, np.float32); bq = np.asarray(bq, np.float32)
    Wk = np.asarray(Wk, np.float32); bk = np.asarray(bk, np.float32)
    Wv = np.asarray(Wv, np.float32); bv = np.asarray(bv, np.float32)
    Wo = np.asarray(Wo, np.float32)

    # safe mask: fully-masked rows unmask key 0 (matches reference)
    mask = mask.copy()
    all_masked = mask.all(axis=1)
    mask[all_masked, 0] = False
    addend = np.where(mask, NEG, np.float32(0.0))       # [B, NK]

    qT = [np.ascontiguousarray(query[b].T).astype(BF16) for b in range(B)]
    kT = [np.ascontiguousarray(key[b].T).astype(BF16) for b in range(B)]
    vT = [np.ascontiguousarray(value[b].T).astype(BF16) for b in range(B)]
    wq_s = (Wq * np.float32(SCALE)).astype(BF16)
    wk_s = Wk.astype(BF16)
    wv_s = Wv.astype(BF16)
    wo_s = Wo.astype(BF16)
    bq_s = (bq * np.float32(SCALE)).astype(np.float32)

    in_maps = []
    for c in range(N_CORES):
        b = c // CORES_PER_B
        h0 = (c % CORES_PER_B) * HPC
        cols = slice(h0 * HD, (h0 + HPC) * HD)
        # exp(bias^T + mask) : [HPC, NK, NQ]; masked keys -> exactly 0
        bT = np.exp(attn_bias[b, h0:h0 + HPC].transpose(0, 2, 1)
                    + addend[b][None, :, None]).astype(BF16)
        in_maps.append({
            "qT": qT[b], "kT": kT[b], "vT": vT[b],
            "wq": np.ascontiguousarray(wq_s[:, cols]),
            "wk": np.ascontiguousarray(wk_s[:, cols]),
            "wv": np.ascontiguousarray(wv_s[:, cols]),
            "wo": np.ascontiguousarray(wo_s[cols, :]),
            "bq": np.ascontiguousarray(bq_s[cols]).reshape(HDC, 1),
            "bk": np.ascontiguousarray(bk[cols]).reshape(HDC, 1),
            "bv": np.ascontiguousarray(bv[cols]).reshape(1, HDC),
            "biasT": np.ascontiguousarray(bT),
        })
    return in_maps


def kernel(query, key, value, attn_bias, key_padding_mask,
           Wq, bq, Wk, bk, Wv, bv, Wo, bo):
    run = get_runner(reps=1)
    in_maps = make_in_maps(query, key, value, attn_bias, key_padding_mask,
                           Wq, bq, Wk, bk, Wv, bv, Wo, bo)
    results = run(in_maps)
    bo = np.asarray(bo, np.float32)
    out = np.zeros((B, NQ, D), np.float32)
    for c in range(N_CORES):
        out[c // CORES_PER_B] += results[c]["part"].astype(np.float32)
    out += bo[None, None, :]
    return out


# revision 23
# speedup vs baseline: 1.0063x; 1.0063x over previous
"""Trainium2 Bass kernel for nn_CrossAttentionWithBias.

Multi-head cross attention with additive bias + key padding mask:
    q = query @ Wq + bq; k = key @ Wk + bk; v = value @ Wv + bv   (per-head split)
    logits = q k^T * hd^-0.5 + attn_bias;  masked keys -> -inf
    out = softmax(logits) @ v, heads merged, @ Wo + bo

Sharding: 8 NeuronCores; core c handles batch b = c//4 and the 4 heads
[4*(c%4), 4*(c%4)+4). Each core computes a partial of the output projection
(its 256 hd-columns' contribution); the host sums the 4 partials per batch
and adds bo.

On-core layout (all "transposed"): activations are stored d-major so the
contraction dim of every matmul is the partition dim:
    qT/kT/vT  [D, n]        (host-transposed, bf16)
    Q^T,K^T   [hd=256, n]   = W.T @ actT     (PE)
    V         [nk, hd]      (natural, PE)    + an all-ones column per head
    S^T       [nk, nq] = K^T.T @ Q^T per head; bias^T folded via exp-decompose
    AV^T      [65, nq] accumulated over 16 k-tiles; row 64 = softmax denom
              (from the ones column) -> reciprocal -> broadcast -> scale
    partial   = AV^T.T @ Wo[hd-rows]         (PE), bf16 out
Softmax skips the max-subtraction: logits here are O(+-10) and masked
entries are -1e30 (exp -> 0 exactly), so exp cannot overflow in f32.

Schedule (vs the 194us baseline): input DMAs are split per d-tile and
issued weights-first on the scalar HWDGE queue so the projections (kt-outer)
trail the DMA stream instead of waiting for it; the attention exp stream
(the ACT-engine wall, ~73us) starts as soon as Q^T/K^T exist; V-projection
passes and AV matmuls of earlier heads are emission-interleaved with S
matmuls of later heads so the in-order PE queue never starves ACT.
"""

import numpy as np

B, NQ, NK, D, H = 2, 1024, 2048, 1024, 16
HD = D // H          # 64
SCALE = HD ** -0.5
N_CORES = 8
CORES_PER_B = N_CORES // B   # 4
HPC = H // CORES_PER_B       # heads per core = 4
HDC = HPC * HD               # hd-columns per core = 256
P = 128
KT_N = NK // P               # 16 k-tiles
DT = D // P                  # 8 d-tiles
QT_N = NQ // P               # 8 q-tiles

BF16 = np.float16
NEG = np.float32(-1e30)
TIMER_NOP = 600          # gpsimd timed-nop cycles per timer tick (1.2 GHz)


# ----------------------------------------------------------------- device ---

def _build_nc(reps: int = 1, timing: bool = False, timer_nop: int = TIMER_NOP):
    # timing=True: big tensors become Internal DRAM (garbage contents; engine
    # timing is data-independent) so per-call host<->device transfer is tiny
    # and wall-clock deltas measure device execution.
    import concourse.bacc as bacc
    import concourse.mybir as mybir
    import concourse.tile as tile

    f32 = mybir.dt.float32
    bf16 = mybir.dt.float16  # "bf16" name kept; fp16 has 11-bit mantissa at same byte width
    Exp = mybir.ActivationFunctionType.Exp
    Ident = mybir.ActivationFunctionType.Identity
    MULT = mybir.AluOpType.mult

    nc = bacc.Bacc("TRN2", target_bir_lowering=False, debug=False,
                   num_devices=N_CORES)

    ikind = "Internal" if timing else "ExternalInput"
    okind = "Internal" if timing else "ExternalOutput"
    qT = nc.dram_tensor("qT", [D, NQ], bf16, kind=ikind).ap()
    kT = nc.dram_tensor("kT", [D, NK], bf16, kind=ikind).ap()
    vT = nc.dram_tensor("vT", [D, NK], bf16, kind=ikind).ap()
    wq = nc.dram_tensor("wq", [D, HDC], bf16, kind=ikind).ap()
    wk = nc.dram_tensor("wk", [D, HDC], bf16, kind=ikind).ap()
    wv = nc.dram_tensor("wv", [D, HDC], bf16, kind=ikind).ap()
    wo = nc.dram_tensor("wo", [HDC, D], bf16, kind=ikind).ap()
    bq = nc.dram_tensor("bq", [HDC, 1], f32, kind=ikind).ap()
    bk = nc.dram_tensor("bk", [HDC, 1], f32, kind=ikind).ap()
    bv = nc.dram_tensor("bv", [1, HDC], f32, kind=ikind).ap()
    biasT = nc.dram_tensor("biasT", [HPC, NK, NQ], bf16, kind=ikind).ap()
    part = nc.dram_tensor("part", [NQ, D], bf16, kind=okind).ap()
    if timing:
        tin = nc.dram_tensor("tin", [1, P], f32, kind="ExternalInput").ap()
        tout = nc.dram_tensor("tout", [1, P], f32, kind="ExternalOutput").ap()

    qTr = qT.rearrange("(t p) n -> p t n", p=P)
    kTr = kT.rearrange("(t p) n -> p t n", p=P)
    vTr = vT.rearrange("(t p) n -> p t n", p=P)

    with tile.TileContext(nc) as tc:
        from contextlib import ExitStack
        with ExitStack() as ctx:
            wpool = ctx.enter_context(tc.tile_pool(name="w", bufs=1))
            # PSUM partition (8 banks total):
            #   sp  2 slots x 2 banks: Q/K projection psums, attention s
            #       tiles, O-proj psums (rotation doubles as s double-buffer)
            #   avp 1 slot  x 2 banks: av accumulators (heads sequential)
            #   vp  2 slots x 1 bank : V-projection, one nk-tile per slot
            sp = ctx.enter_context(tc.tile_pool(name="sp", bufs=2, space="PSUM"))
            avp = ctx.enter_context(tc.tile_pool(name="avp", bufs=1, space="PSUM"))
            vp = ctx.enter_context(tc.tile_pool(name="vp", bufs=2, space="PSUM"))
            apool = ctx.enter_context(tc.tile_pool(name="a", bufs=1))
            dpool = ctx.enter_context(tc.tile_pool(name="d", bufs=2))
            bias_pool = ctx.enter_context(tc.tile_pool(name="bias", bufs=4))
            e_pool = ctx.enter_context(tc.tile_pool(name="e", bufs=3))
            et_pool = ctx.enter_context(tc.tile_pool(name="et", bufs=4))
            small = ctx.enter_context(tc.tile_pool(name="small", bufs=2))
            ostage = ctx.enter_context(tc.tile_pool(name="ostage", bufs=4))

            if timing:
                # On-device timer: the sync sequencer spins on a timed-nop
                # loop, polling an SBUF flag via an aliased handle (invisible
                # to the dep tracker, so the spin doesn't deadlock against
                # the final flag write). count * period = kernel span.
                tmr_pool = ctx.enter_context(tc.tile_pool(name="tmr", bufs=1))
                cnt_sb = tmr_pool.tile([1, 2], mybir.dt.int32, tag="cnt")
                wflag = nc.alloc_sbuf_tensor("wflag", [1, 1], f32)
                waddr = nc.lookup_mloc(wflag).addr
                rflag = nc.alloc_sbuf_tensor_at(
                    "rflag", [1, 1], mybir.dt.int32, offset=waddr)
                with tc.tile_critical():
                    g = nc.sync
                    g.store(rflag.ap()[0:1, 0:1], 0)
                    with g.register("tmr_cnt") as cnt, \
                         g.register("tmr_f") as fr, \
                         g.register("tmr_c") as cr:
                        g.reg_mov(cnt, 0)
                        g.reg_mov(cr, 1)
                        with g.While(lambda: cr):
                            g.nop(cycle_cnt=timer_nop)
                            g.reg_alu(cnt, cnt, 1, mybir.AluOpType.add)
                            g.reg_load(fr, rflag.ap()[0:1, 0:1])
                            g.reg_alu(cr, fr, 0, mybir.AluOpType.is_equal)
                        g.store(cnt_sb[0:1, 0:1], cnt)

            # V (+ 64 ones cols -> replicated softmax denom): [P, kt, h, 128]
            # allocated once; ones written once, V cols rewritten per rep
            V_sb = apool.tile([P, KT_N, HPC, 2 * HD], bf16, tag="V")
            nc.vector.memset(V_sb[:, :, :, HD:], 1.0)

            if timing:
                # Internal DRAM holds garbage; fill with benign values so the
                # compute stays finite (exp(0)=1, biasT=1 -> denom=NK).
                zfill = apool.tile([P, 2048], bf16, tag="zfill")
                nc.vector.memset(zfill[:], 0.0)
                ofill = apool.tile([P, 2048], bf16, tag="ofill")
                nc.vector.memset(ofill[:], 1.0)
                zf32 = apool.tile([P, 2 * HDC], f32, tag="zf32")
                nc.vector.memset(zf32[:], 0.0)
                for t, width in ((qT, NQ), (kT, NK), (vT, NK)):
                    r = t.rearrange("(t p) n -> p t n", p=P)
                    for i in range(DT):
                        for j in range(width // 2048):
                            nc.scalar.dma_start(
                                r[:, i, j * 2048:(j + 1) * 2048],
                                zfill[:, 0:2048])
                        if width < 2048:
                            nc.scalar.dma_start(r[:, i, :], zfill[:, 0:width])
                for t in (wq, wk, wv):
                    r = t.rearrange("(t p) m -> p t m", p=P)
                    for i in range(DT):
                        nc.scalar.dma_start(r[:, i, :], zfill[:, 0:HDC])
                r = wo.rearrange("(t p) n -> p t n", p=P)
                for i in range(HDC // P):
                    nc.scalar.dma_start(r[:, i, :], zfill[:, 0:D])
                nc.scalar.dma_start(bq.rearrange("(t p) o -> p t o", p=P),
                                  zf32[:, 0:2].bitcast(f32)[:, 0:2, None])
                nc.scalar.dma_start(bk.rearrange("(t p) o -> p t o", p=P),
                                  zf32[:, 0:2].bitcast(f32)[:, 0:2, None])
                nc.scalar.dma_start(bv[:], zf32[0:1, 0:HDC])
                for h in range(HPC):
                    r = biasT[h].rearrange("(t p) q -> p t q", p=P)
                    for i in range(KT_N):
                        nc.scalar.dma_start(r[:, i, :], ofill[:, 0:NQ])

            # Input-stream DMA engine: SP's HWDGE queue when it's free (the
            # harness path), scalar's when SP runs the spin timer.  Either
            # way FIFO per engine, so arrival order == consumption order.
            dma = nc.scalar if timing else nc.sync

            for _ in range(reps):
                # ---- input stream: wv,bv,vT | wq,bq,qT | wk,bk,kT, split
                # per d-tile so the kt-outer projections trail arrivals;
                # weights ride just ahead of the activations they process.
                wv_sb = wpool.tile([P, DT, HDC], bf16, tag="wv")
                dma.dma_start(wv_sb[:], wv.rearrange("(t p) m -> p t m", p=P))
                bv_sb = wpool.tile([P, HDC], f32, tag="bv")
                dma.dma_start(bv_sb[:], bv[:].to_broadcast((P, HDC)))
                bv_bc = bv_sb[:].rearrange("p (h e) -> p h e", h=HPC)
                vT_sb = apool.tile([P, DT, NK], bf16, tag="vT")
                for t in range(DT):
                    dma.dma_start(vT_sb[:, t, :], vTr[:, t, :])
                wq_sb = wpool.tile([P, DT, HDC], bf16, tag="wq")
                dma.dma_start(wq_sb[:], wq.rearrange("(t p) m -> p t m", p=P))
                bq_sb = wpool.tile([P, HDC // P, 1], f32, tag="bq")
                dma.dma_start(bq_sb[:], bq.rearrange("(t p) o -> p t o", p=P))
                qT_sb = apool.tile([P, DT, NQ], bf16, tag="qT")
                for t in range(DT):
                    dma.dma_start(qT_sb[:, t, :], qTr[:, t, :])
                wk_sb = wpool.tile([P, DT, HDC], bf16, tag="wk")
                dma.dma_start(wk_sb[:], wk.rearrange("(t p) m -> p t m", p=P))
                bk_sb = wpool.tile([P, HDC // P, 1], f32, tag="bk")
                dma.dma_start(bk_sb[:], bk.rearrange("(t p) o -> p t o", p=P))
                kT_sb = apool.tile([P, DT, NK], bf16, tag="kT")
                for t in range(DT):
                    dma.dma_start(kT_sb[:, t, :], kTr[:, t, :])
                wo_sb = wpool.tile([P, HDC // P, D], bf16, tag="wo")

                # QT/KT: [hd, n] as [P, ht, n];  AVT result shares layout
                QT_sb = dpool.tile([P, 2, NQ], bf16, tag="QT")
                KT_sb = dpool.tile([P, 2, NK], bf16, tag="KT")
                AVT_sb = dpool.tile([P, 2, NQ], bf16, tag="AVT")

                # V-projection of one nk-tile mt into a 1-bank vp slot;
                # d_tiles=None -> full d-inner contraction in one call.
                psV = [None] * KT_N

                def v_mm(mt, d_tiles=None, evac=False):
                    if psV[mt] is None:
                        psV[mt] = vp.tile([P, 512], f32, tag="psV",
                                          name="psV")
                    for kt in (range(DT) if d_tiles is None else d_tiles):
                        nc.tensor.matmul(
                            psV[mt][:, 0:HDC],
                            lhsT=vT_sb[:, kt, mt * P:(mt + 1) * P],
                            rhs=wv_sb[:, kt, :],
                            start=(kt == 0), stop=(kt == DT - 1))
                    if evac or d_tiles is None:
                        nc.vector.tensor_add(
                            V_sb[:, mt, :, 0:64],
                            psV[mt][:, 0:HDC].rearrange("p (h e) -> p h e",
                                                        h=HPC),
                            bv_bc)

                # ---- V nk-tiles 0,1 d-outer (trail the vT stream),
                # then 2..15 d-inner (vT resident); all pre-attention ----
                for kt in range(DT):
                    v_mm(0, d_tiles=[kt], evac=(kt == DT - 1))
                    v_mm(1, d_tiles=[kt], evac=(kt == DT - 1))
                for mt in range(2, KT_N):
                    v_mm(mt)

                # ---- Q projection, kt-outer (trails the qT DMA stream) ----
                psQ = [sp.tile([P, NQ], f32, tag="ps", name="psQ") for _ in range(2)]
                for kt in range(DT):
                    for ht in range(2):
                        for ch in range(2):
                            csl = slice(ch * 512, (ch + 1) * 512)
                            nc.tensor.matmul(
                                psQ[ht][:, csl],
                                lhsT=wq_sb[:, kt, ht * P:(ht + 1) * P],
                                rhs=qT_sb[:, kt, csl],
                                start=(kt == 0), stop=(kt == DT - 1))
                for ht in range(2):
                    nc.vector.tensor_scalar(
                        QT_sb[:, ht, :], psQ[ht][:], bq_sb[:, ht, :], None,
                        mybir.AluOpType.add)

                # ---- K projection, nk-quarter-major in 1-bank vp slots.
                # Needs the full kT stream (d-contraction), so quarter 0 of
                # ht0 gates the first S; the other quarters are injected
                # into attention loops ahead of their first use. ----
                psKq_live = {}

                def k_quarter(ht, q, part_i=None):
                    # part_i 0/1 splits the 8-MM contraction into two bursts
                    # (injected at adjacent attention steps to stay inside
                    # the per-step PE slack); None = both at once.
                    qsl = slice(q * 512, (q + 1) * 512)
                    if part_i in (0, None):
                        psKq_live[(ht, q)] = vp.tile([P, 512], f32,
                                                     tag="psV", name="psK")
                    psKq = psKq_live[(ht, q)]
                    tiles = (range(DT) if part_i is None else
                             range(part_i * DT // 2, (part_i + 1) * DT // 2))
                    for kt in tiles:
                        nc.tensor.matmul(
                            psKq[:], lhsT=wk_sb[:, kt, ht * P:(ht + 1) * P],
                            rhs=kT_sb[:, kt, qsl],
                            start=(kt == 0), stop=(kt == DT - 1))
                    if part_i in (1, None):
                        nc.vector.tensor_scalar(
                            KT_sb[:, ht, qsl], psKq[:], bk_sb[:, ht, :],
                            None, mybir.AluOpType.add)

                k_quarter(0, 0)

                # Gate the gpsimd-side DMAs (eb stream, wo) behind the kT
                # DMAs so their prefetch doesn't steal HBM bandwidth from
                # the input stream that gates the first exp.  The Tile
                # scheduler runs any *ready* instruction past a blocked one,
                # so ordering alone is not enough: each gated DMA's buffer
                # first gets a corner written from gate_sb (which depends on
                # kT), making the DMA itself transitively depend on kT.
                gate_sb = small.tile([1, 8], bf16, tag="gate")
                nc.gpsimd.tensor_copy(gate_sb[:], kT_sb[0:1, DT - 1, 0:8])

                def gated_pool_dma(dst_tile, corner, src_ap):
                    nc.gpsimd.tensor_copy(corner, gate_sb[0:1, 0:1])
                    nc.gpsimd.dma_start(dst_tile, src_ap)

                gated_pool_dma(wo_sb[:], wo_sb[0:1, 0, 0:1],
                               wo.rearrange("(t p) n -> p t n", p=P))

                # ---- attention ----
                # biasT input holds exp(bias^T + mask) ("EB"); logits
                # exp-decompose: exp(S+B) = exp(S) * EB, so the bias lands
                # as a 2-byte DVE multiply (2x mode) off the PSUM path.
                # Per head, per k-tile: S (PE) -> exp (ACT) -> *EB (DVE) ->
                # AV accumulate (PE).  s tiles rotate through sp's 2 slots,
                # so PE runs one k-tile ahead of the exp; the ACT exp stream
                # (the wall) never waits.  V nk-tiles 8-15 (vT half1, which
                # lands mid-attention) are injected into h0's loop at spots
                # where the PE would idle anyway, ahead of their AV use.
                eb_tiles = {}

                def attn_head(h, inject):
                    ht, hp = h // 2, (h % 2) * HD
                    av = avp.tile([P, NQ], f32, tag="av")
                    for kt in range(KT_N):
                        if kt % 2 == 0:
                            eb = bias_pool.tile([P, 2, NQ], bf16, tag="bias")
                            gated_pool_dma(
                                eb[:], eb[0:1, 0, 0:1],
                                biasT[h].rearrange("(t p) q -> p t q",
                                                   p=P)[:, kt:kt + 2, :])
                            eb_tiles[h] = eb
                        s = sp.tile([P, NQ], f32, tag="ps", name="s")
                        ksl = slice(kt * P, (kt + 1) * P)
                        for ch in range(2):
                            csl = slice(ch * 512, (ch + 1) * 512)
                            nc.tensor.matmul(
                                s[:, csl],
                                lhsT=KT_sb[hp:hp + HD, ht, ksl],
                                rhs=QT_sb[hp:hp + HD, ht, csl],
                                start=True, stop=True)
                        e0 = e_pool.tile([P, NQ], bf16, tag="e")
                        nc.scalar.activation(e0[:], s[:], Exp)
                        et = et_pool.tile([P, NQ], bf16, tag="et")
                        nc.vector.tensor_tensor(
                            et[:], e0[:], eb_tiles[h][:, kt % 2, :], op=MULT)
                        for ch in range(2):
                            csl = slice(ch * 512, (ch + 1) * 512)
                            nc.tensor.matmul(
                                av[:, csl],
                                lhsT=V_sb[:, kt, h, :],
                                rhs=et[:, csl],
                                start=(kt == 0), stop=(kt == KT_N - 1))
                        for fn in inject.get(kt, ()):
                            fn()
                    rec_bc = small.tile([HD, NQ], f32, tag="rec_bc")
                    nc.vector.reciprocal(rec_bc[:], av[HD:2 * HD, :])
                    nc.vector.tensor_tensor(
                        AVT_sb[hp:hp + HD, ht, :], av[0:HD, :],
                        rec_bc[:], op=MULT)

                # Injected K-quarter work, placed ahead of its first
                # consumer: ht0 quarters 1-3 before S_h0 k-tiles 4/8/12;
                # ht1 quarters before exp_h2 k-tiles 0/4/8/12.
                attn_head(0, {
                    0: (lambda: k_quarter(0, 1, 0),),
                    1: (lambda: k_quarter(0, 1, 1),),
                    2: (lambda: k_quarter(0, 2, 0),),
                    3: (lambda: k_quarter(0, 2, 1),),
                    4: (lambda: k_quarter(0, 3, 0),),
                    5: (lambda: k_quarter(0, 3, 1),),
                })
                attn_head(1, {
                    0: (lambda: k_quarter(1, 0, 0),),
                    1: (lambda: k_quarter(1, 0, 1),),
                    4: (lambda: k_quarter(1, 1, 0),),
                    5: (lambda: k_quarter(1, 1, 1),),
                })
                attn_head(2, {
                    1: (lambda: k_quarter(1, 2, 0),),
                    2: (lambda: k_quarter(1, 2, 1),),
                    5: (lambda: k_quarter(1, 3, 0),),
                    6: (lambda: k_quarter(1, 3, 1),),
                })
                attn_head(3, {})

                # ---- output projection (partial), bf16 out ----
                last_ot = None
                for mt in range(QT_N):
                    ps = sp.tile([P, D], f32, tag="ps", name="ops")
                    for ch in range(D // 512):
                        csl = slice(ch * 512, (ch + 1) * 512)
                        for kt in range(HDC // P):
                            nc.tensor.matmul(
                                ps[:, csl],
                                lhsT=AVT_sb[:, kt, mt * P:(mt + 1) * P],
                                rhs=wo_sb[:, kt, csl],
                                start=(kt == 0), stop=(kt == HDC // P - 1))
                    ot = ostage.tile([P, D], bf16, tag="ot")
                    nc.vector.tensor_copy(ot[:, 0:512], ps[:, 0:512])
                    nc.scalar.copy(ot[:, 512:D], ps[:, 512:D])
                    nc.gpsimd.dma_start(
                        part[mt * P:(mt + 1) * P, :], ot[:])
                    last_ot = ot
            if timing:
                # tiny externally-visible dependency so the compute isn't dead
                tsb = small.tile([1, P], f32, tag="tsb")
                nc.scalar.dma_start(tsb[:], tin[:])
                nc.vector.tensor_add(tsb[:], tsb[:], last_ot[0:1, 0:P])
                # stop the timer: write nonzero to the flag (depends on tsb)
                nc.vector.tensor_scalar(
                    wflag.ap()[0:1, 0:1], tsb[0:1, 0:1], 0.0, 1.0,
                    mybir.AluOpType.mult, mybir.AluOpType.add)
                nc.scalar.dma_start(tout[:], tsb[:])
                nc.scalar.dma_start(tout[0:1, 0:2].bitcast(mybir.dt.int32),
                                    cnt_sb[0:1, 0:2])

    nc.compile()
    return nc


# ----------------------------------------------------------------- runner ---

def _make_runner(nc, n_cores):
    import jax
    from jax.sharding import Mesh, PartitionSpec
    from jax.experimental.shard_map import shard_map
    import concourse.mybir as mybir
    from concourse import bass2jax
    from concourse.bass2jax import _bass_exec_p, install_neuronx_cc_hook

    install_neuronx_cc_hook()
    partition_name = nc.partition_id_tensor.name if nc.partition_id_tensor else None
    dbg_name = nc.dbg_addr.name if nc.dbg_addr is not None else None
    in_names, out_names, out_avals, zero_outs = [], [], [], []
    for alloc in nc.m.functions[0].allocations:
        if not isinstance(alloc, mybir.MemoryLocationSet):
            continue
        name = alloc.memorylocations[0].name
        if alloc.kind == "ExternalInput":
            if name not in (partition_name, dbg_name):
                in_names.append(name)
        elif alloc.kind == "ExternalOutput":
            out_names.append(name)
            shape = tuple(alloc.tensor_shape)
            dtype = mybir.dt.np(alloc.dtype)
            out_avals.append(jax.core.ShapedArray(shape, dtype))
            zero_outs.append(np.zeros(shape, dtype))
    n_params = len(in_names)
    all_in_names = list(in_names) + list(out_names)
    if dbg_name is not None:
        all_in_names.append(dbg_name)
    if partition_name is not None:
        all_in_names.append(partition_name)

    def _body(*args):
        operands = list(args)
        if dbg_name is not None:
            operands.append(np.zeros((1, 2), np.uint32))
        if partition_name is not None:
            operands.append(bass2jax.partition_id_tensor())
        outs = _bass_exec_p.bind(
            *operands,
            out_avals=tuple(out_avals),
            in_names=tuple(all_in_names),
            out_names=tuple(out_names),
            lowering_input_output_aliases=(),
            sim_require_finite=True,
            sim_require_nnan=True,
            nc=nc,
        )
        return tuple(outs)

    devices = jax.devices()[:n_cores]
    mesh = Mesh(np.asarray(devices), ("core",))
    in_specs = (PartitionSpec("core"),) * (n_params + len(out_names))
    out_specs = (PartitionSpec("core"),) * len(out_names)
    sharded = jax.jit(
        shard_map(_body, mesh=mesh, in_specs=in_specs, out_specs=out_specs,
                  check_rep=False),
        keep_unused=True,
    )

    def run(in_maps):
        concat_in = [
            np.concatenate([np.asarray(in_maps[c][n]) for c in range(n_cores)],
                           axis=0)
            for n in in_names
        ]
        concat_zeros = [
            np.zeros((n_cores * z.shape[0], *z.shape[1:]), z.dtype)
            for z in zero_outs
        ]
        out = sharded(*concat_in, *concat_zeros)
        return [
            {name: np.asarray(out[i]).reshape(n_cores, *out_avals[i].shape)[c]
             for i, name in enumerate(out_names)}
            for c in range(n_cores)
        ]

    run.sharded = sharded
    run.in_names = in_names
    run.out_names = out_names
    run.out_avals = out_avals
    run.zero_outs = zero_outs
    return run


_CACHE = {}


def get_runner(reps: int = 1, timing: bool = False, timer_nop: int = TIMER_NOP):
    key = (reps, timing, timer_nop)
    if key not in _CACHE:
        nc = _build_nc(reps, timing=timing, timer_nop=timer_nop)
        _CACHE[key] = _make_runner(nc, N_CORES)
    return _CACHE[key]


# ------------------------------------------------------------------- host ---

def make_in_maps(query, key, value, attn_bias, key_padding_mask,
                 Wq, bq, Wk, bk, Wv, bv, Wo, bo):
    query = np.asarray(query, np.float32)
    key = np.asarray(key, np.float32)
    value = np.asarray(value, np.float32)
    attn_bias = np.asarray(attn_bias, np.float32)
    mask = np.asarray(key_padding_mask, bool)
    Wq = np.asarray(Wq, np.float32); bq = np.asarray(bq, np.float32)
    Wk = np.asarray(Wk, np.float32); bk = np.asarray(bk, np.float32)
    Wv = np.asarray(Wv, np.float32); bv = np.asarray(bv, np.float32)
    Wo = np.asarray(Wo, np.float32)

    # safe mask: fully-masked rows unmask key 0 (matches reference)
    mask = mask.copy()
    all_masked = mask.all(axis=1)
    mask[all_masked, 0] = False
    addend = np.where(mask, NEG, np.float32(0.0))       # [B, NK]

    qT = [np.ascontiguousarray(query[b].T).astype(BF16) for b in range(B)]
    kT = [np.ascontiguousarray(key[b].T).astype(BF16) for b in range(B)]
    vT = [np.ascontiguousarray(value[b].T).astype(BF16) for b in range(B)]
    wq_s = (Wq * np.float32(SCALE)).astype(BF16)
    wk_s = Wk.astype(BF16)
    wv_s = Wv.astype(BF16)
    wo_s = Wo.astype(BF16)
    bq_s = (bq * np.float32(SCALE)).astype(np.float32)

    in_maps = []
    for c in range(N_CORES):
        b = c // CORES_PER_B
        h0 = (c % CORES_PER_B) * HPC
        cols = slice(h0 * HD, (h0 + HPC) * HD)
        # exp(bias^T + mask) : [HPC, NK, NQ]; masked keys -> exactly 0
        bT = np.exp(attn_bias[b, h0:h0 + HPC].transpose(0, 2, 1)
                    + addend[b][None, :, None]).astype(BF16)
        in_maps.append({
            "qT": qT[b], "kT": kT[b], "vT": vT[b],
            "wq": np.ascontiguousarray(wq_s[:, cols]),
            "wk": np.ascontiguousarray(wk_s[:, cols]),
            "wv": np.ascontiguousarray(wv_s[:, cols]),
            "wo": np.ascontiguousarray(wo_s[cols, :]),
            "bq": np.ascontiguousarray(bq_s[cols]).reshape(HDC, 1),
            "bk": np.ascontiguousarray(bk[cols]).reshape(HDC, 1),
            "bv": np.ascontiguousarray(bv[cols]).reshape(1, HDC),
            "biasT": np.ascontiguousarray(bT),
        })
    return in_maps


def kernel(query, key, value, attn_bias, key_padding_mask,
           Wq, bq, Wk, bk, Wv, bv, Wo, bo):
    run = get_runner(reps=1)
    in_maps = make_in_maps(query, key, value, attn_bias, key_padding_mask,
                           Wq, bq, Wk, bk, Wv, bv, Wo, bo)
    results = run(in_maps)
    bo = np.asarray(bo, np.float32)
    out = np.zeros((B, NQ, D), np.float32)
    for c in range(N_CORES):
        out[c // CORES_PER_B] += results[c]["part"].astype(np.float32)
    out += bo[None, None, :]
    return out
